# revision 4
# baseline (speedup 1.0000x reference)
"""Trainium2 Bass kernel for nn_Attention_3032246911698 (sparse_attention).

Computes, per batch row b:
    score_dec = v[0] @ W_v.T + attn_b                      # [B, H]
    score_enc = einsum('ble,he->blh', encoder_out, W_e)    # [B, L, H]
    en        = tanh(score_dec[:,None,:] + score_enc)      # [B, L, H]
    att       = einsum('blh,h->bl', en, v_w[0])            # [B, L]
    att       = where(mask == 0, -1e10, att)
    out       = softmax(att, axis=1)                       # [B, L]

Sharding: data-parallel over batch B=16 across 8 NeuronCores (2 rows each).
Weights replicated.  No cross-core communication.

v3 design (fp8 DoubleRow; v2 measured 163us, bf16 v1 325-360us):
  - score_enc in fp8e4 (TRN E4M3) with perf_mode=DoubleRow: 2 fp8 weights
    per PE cell, K=256 per matmul.  v2 measured the DR stream at 222ns per
    LDW+MM pair (the pure N=512 streaming floor is 213ns), i.e. ~96% PE
    efficiency -- so v3 keeps v2's matmul structure untouched.  All
    weights (W_e, W_v, dec) are fp8 with a x64 host pre-scale (their
    ~N(0,0.02^2) entries would be subnormal in e4m3); the 1/64 rides the
    ACT scale operand.  encoder_out is fp8-quantized AND pre-transposed on
    host into the exact SBUF layout, so the device feed is plain 1MB DMAs
    with 8KB/partition lines.  Simulated rel err 1.46e-2 (gate 2e-2).
  - v3 vs v2 is about the 16.5us startup and 7.7us tail that the v2 trace
    showed: the NEFF preamble is ~6us and DMA rings only move bytes from
    ~8.4us, so every startup byte matters.  score_dec's inputs (wv, dec)
    now go fp8 and are emitted FIRST on the SWDGE queues (in v2 they sat
    behind wpair in queue order, stalling the PE FIFO head until 16.4us);
    wpair is split between SWDGE and the Activation engine's HWDGE ring;
    all 8 enc chunks are loaded up-front into one 64KB/partition SBUF
    tile, alternating the Sync and Activation HWDGE rings, which also
    removes v2's 4.1us chunk-0 wait.
  - en stays bf16 (fp8 there pushes rel err to ~0.0204, over the gate);
    att keeps the 8 K=128 matmuls + one K=1 mask matmul (M=1, LDWEIGHTS
    ~free, adds (mask-1)*1e10 so masked lanes exp to exactly 0).
  - softmax has no max pass (|logits| <= ~2): per-chunk ACT Exp off PSUM
    emits partial sums via accum_out; the tail is sum(4) + reciprocal +
    a half-row scale on Scalar (ACT Copy w/ scale AP) in parallel with a
    half-row scale on Vector, stores split over the Act-HWDGE and SWDGE.
Hardware constraints baked in: walrus accepts ONE sync-wait per
instruction (hence bacc.Bacc + ACT Identity bias adds); fp32 matmul is 4x
slower than bf16 and fp8 without DoubleRow runs at bf16 speed; DoubleRow
operand APs are [K=128, 2, free] with 16B-aligned pair strides, pair =
adjacent ec-chunks (e = ec*128 + p), matching tile_matmul production use
(score_dec stays non-DR: its rhs pair stride BC*1B would break the 16B
alignment rule).
"""

import os
import sys

import numpy as np

for _p in ("/opt/trn_rl_repo", "/root/.axon_site/_ro/trn_rl_repo"):
    if os.path.isdir(_p) and _p not in sys.path:
        sys.path.append(_p)

import concourse.bass as bass  # noqa: F401  (engine types referenced via nc)
import concourse.mybir as mybir
import concourse.tile as tile
from concourse import bacc
from concourse.bass_utils import run_bass_kernel_spmd

import ml_dtypes

BF16 = ml_dtypes.bfloat16
E4M3 = ml_dtypes.float8_e4m3  # TRN FP8_EXP4: max normal 240 (not the fn variant)

F32 = mybir.dt.float32
BF = mybir.dt.bfloat16
F8 = mybir.dt.float8e4

N_CORES = 8
B, L, H = 16, 2048, 1024
E = 2 * H
BC = B // N_CORES          # 2 batch rows per core
TCH = 512                  # tokens per t-chunk (one PSUM bank of f32)
NCHUNK = L // TCH          # 4 t-chunks per batch row
NSLOT = BC * NCHUNK        # 8 chunk slots, all resident in SBUF
EC = E // 128              # 16 e-chunks of 128
EP = EC // 2               # 8 DoubleRow e-pairs per contraction
KC = H // 128              # 8 h-chunks
W_SCALE = 64.0             # host premultiplier on W_e/W_v/dec before fp8


def build_nc():
    nc = bacc.Bacc(num_swdge_queues=4)

    # encT[b, p, ci, ec, t] = fp8(enc[b, ci*TCH + t, ec*128 + p])
    encP = nc.declare_dram_parameter(
        "encT", [BC, 128, NCHUNK, EC, TCH], F8, isOutput=False)
    # wpair[p, ec, h] = fp8(64 * W_e[h, ec*128 + p])
    wpairP = nc.declare_dram_parameter("wpair", [128, EC, H], F8, isOutput=False)
    # wv[p, hic, ho] = fp8(64 * W_v[ho, hic*128 + p])
    wvP = nc.declare_dram_parameter("wv", [128, KC, H], F8, isOutput=False)
    # decT[p, hic, b] = fp8(v[0, b, hic*128 + p])
    decP = nc.declare_dram_parameter("decT", [128, KC, BC], F8, isOutput=False)
    bP = nc.declare_dram_parameter("attn_bT", [128, KC, 1], F32, isOutput=False)
    vwP = nc.declare_dram_parameter("v_wT", [128, KC, 1], BF, isOutput=False)
    # (mask-1)*1e10 in bf16: 0 where kept, ~-1e10 where masked
    maskP = nc.declare_dram_parameter("maskadd", [BC, L], BF, isOutput=False)
    out = nc.declare_dram_parameter("out", [BC, L], F32, isOutput=True)

    TANH = mybir.ActivationFunctionType.Tanh
    EXP = mybir.ActivationFunctionType.Exp
    IDENT = mybir.ActivationFunctionType.Identity
    DR = mybir.MatmulPerfMode.DoubleRow

    with tile.TileContext(nc) as tc:
        with (
            tc.tile_pool(name="consts", bufs=1) as consts,
            tc.tile_pool(name="en", bufs=2) as en_pool,
            tc.tile_pool(name="rowbig", bufs=2) as rowbig_pool,
            tc.tile_pool(name="rowsmall", bufs=2) as rowsmall_pool,
            tc.tile_pool(name="psum_score", bufs=4, space="PSUM") as score_psum,
            tc.tile_pool(name="psum_att", bufs=2, space="PSUM") as att_psum,
        ):
            # ---- weights / inputs: startup-latency-ordered DMAs ------------
            # SWDGE (gpsimd) queue, highest priority first: score_dec's
            # inputs unblock the PE FIFO head.
            ones1 = consts.tile([1, 1], BF)
            nc.gpsimd.memset(ones1, 1.0)

            dec_tile = consts.tile([128, KC, BC], F8)
            nc.gpsimd.dma_start(dec_tile, decP[:, :, :])
            b_tile = consts.tile([128, KC, 1], F32)
            nc.gpsimd.dma_start(b_tile, bP[:, :, :])
            vw_tile = consts.tile([128, KC, 1], BF)
            nc.gpsimd.dma_start(vw_tile, vwP[:, :, :])

            wv_tile = consts.tile([128, KC, H], F8)
            for s in range(2):
                nc.gpsimd.dma_start(
                    wv_tile[:, :, s * 512:(s + 1) * 512],
                    wvP[:, :, s * 512:(s + 1) * 512])

            # wpair: half on SWDGE, half on the Activation HWDGE ring.
            wp_tile = consts.tile([128, EC, H], F8)
            for s in range(2):
                nc.gpsimd.dma_start(
                    wp_tile[:, s * 4:(s + 1) * 4, :], wpairP[:, s * 4:(s + 1) * 4, :])
            for s in range(2, 4):
                nc.scalar.dma_start(
                    wp_tile[:, s * 4:(s + 1) * 4, :], wpairP[:, s * 4:(s + 1) * 4, :])

            maskbs = []
            for b in range(BC):
                mb_t = rowsmall_pool.tile([1, L], BF, tag=f"maskb{b}")
                nc.gpsimd.dma_start(mb_t, maskP[b:b + 1, :])
                maskbs.append(mb_t)

            # All 8 enc chunks up-front, alternating Sync / Activation HWDGE.
            enc_tile = consts.tile([128, NSLOT, EC, TCH], F8)
            for slot in range(NSLOT):
                b, ci = divmod(slot, NCHUNK)
                eng = nc.sync if slot % 2 == 0 else nc.scalar
                eng.dma_start(enc_tile[:, slot, :, :], encP[b, :, ci, :, :])

            # ---- score_dec = dec @ W_v.T + attn_b, stored transposed -------
            # sd_tile[:, hoc, b] = (sum_hi 64*W_v.T[hi,ho] * dec[hi,b])/64 + b[ho]
            sd_tile = consts.tile([128, KC, BC], F32)
            for hoc in range(KC):
                ps_sd = att_psum.tile([128, BC], F32, tag="sdps")
                for hic in range(KC):
                    nc.tensor.matmul(
                        ps_sd,
                        lhsT=wv_tile[:, hic, hoc * 128:(hoc + 1) * 128],
                        rhs=dec_tile[:, hic, :],
                        start=(hic == 0),
                        stop=(hic == KC - 1),
                    )
                # ACT (not DVE tensor_scalar): TensorScalarPtr carries only one
                # sync-wait slot and this op needs two.
                nc.scalar.activation(sd_tile[:, hoc, :], ps_sd, IDENT,
                                     bias=b_tile[:, hoc, :], scale=1.0 / W_SCALE)

            # ---- main loop --------------------------------------------------
            for b in range(BC):
                exps = rowbig_pool.tile([1, L], F32, tag="exps")
                partials = rowsmall_pool.tile([1, NCHUNK], F32, tag="partials")
                for ci in range(NCHUNK):
                    t0 = ci * TCH
                    slot = b * NCHUNK + ci
                    encT = enc_tile[:, slot, :, :]

                    en_big = en_pool.tile([128, KC, TCH], BF, tag="en_big")
                    for hc in range(KC):
                        ps_score = score_psum.tile([128, TCH], F32, tag="ps_score")
                        for ep in range(EP):
                            nc.tensor.matmul(
                                ps_score,
                                lhsT=wp_tile[:, 2 * ep:2 * ep + 2,
                                             hc * 128:(hc + 1) * 128],
                                rhs=encT[:, 2 * ep:2 * ep + 2, :],
                                start=(ep == 0),
                                stop=(ep == EP - 1),
                                perf_mode=DR,
                            )
                        nc.scalar.activation(
                            en_big[:, hc, :], ps_score, TANH,
                            bias=sd_tile[:, hc, b:b + 1], scale=1.0 / W_SCALE,
                        )

                    ps_att = att_psum.tile([1, TCH], F32, tag="attps")
                    for hc in range(KC):
                        nc.tensor.matmul(
                            ps_att,
                            lhsT=vw_tile[:, hc, :],
                            rhs=en_big[:, hc, :],
                            start=(hc == 0),
                            stop=False,
                        )
                    # += (mask-1)*1e10 as a K=1 rank-1 update
                    nc.tensor.matmul(
                        ps_att, lhsT=ones1, rhs=maskbs[b][:, t0:t0 + TCH],
                        start=False, stop=True,
                    )
                    # exp straight off PSUM; |logits| <= ~2 so no max pass,
                    # masked lanes underflow to exactly 0.  accum_out gives
                    # this chunk's partial sum for free.
                    nc.scalar.activation(
                        exps[:, t0:t0 + TCH], ps_att, EXP,
                        accum_out=partials[:, ci:ci + 1],
                    )

                # ---- normalize: sum partials, reciprocal, scale, store -----
                total = rowsmall_pool.tile([1, 1], F32, tag="total")
                nc.vector.reduce_sum(total, partials[:, 0:NCHUNK],
                                     axis=mybir.AxisListType.X)
                rcp = rowsmall_pool.tile([1, 1], F32, tag="rcp")
                nc.vector.reciprocal(rcp, total)
                # split the row: Scalar scales+stores the low half while
                # Vector scales the high half (stored via SWDGE).
                oh0 = rowbig_pool.tile([1, L // 2], F32, tag="oh0")
                oh1 = rowbig_pool.tile([1, L // 2], F32, tag="oh1")
                nc.scalar.mul(oh0, exps[:, 0:L // 2], rcp[:, :])
                nc.vector.tensor_scalar_mul(oh1, exps[:, L // 2:L], rcp[:, :])
                nc.scalar.dma_start(out[b:b + 1, 0:L // 2], oh0)
                nc.gpsimd.dma_start(out[b:b + 1, L // 2:L], oh1)

    nc.finalize()
    return nc


_NC_CACHE = None


def _get_nc():
    global _NC_CACHE
    if _NC_CACHE is None:
        _NC_CACHE = build_nc()
    return _NC_CACHE


def prepare_in_maps(encoder_out, mask, v, attn_w, attn_b, v_w):
    enc = np.asarray(encoder_out, dtype=np.float32)
    enc_q = np.clip(enc, -240.0, 240.0).astype(E4M3)          # [B, L, E]

    attn_w = np.asarray(attn_w, dtype=np.float32)
    W_v = attn_w[:, :H]                                        # [H, H]
    W_e = attn_w[:, H:]                                        # [H, E]
    wpair = np.ascontiguousarray(                              # [128, EC, H]
        np.clip(W_e.T * W_SCALE, -240.0, 240.0)
        .astype(E4M3).reshape(EC, 128, H).transpose(1, 0, 2))
    wv = np.ascontiguousarray(                                 # [128, KC, H]
        np.clip(W_v.T * W_SCALE, -240.0, 240.0)
        .astype(E4M3).reshape(KC, 128, H).transpose(1, 0, 2))

    dec = np.asarray(v, dtype=np.float32)[0]                   # [B, H]
    bT = np.ascontiguousarray(                                 # [128, KC, 1]
        np.asarray(attn_b, dtype=np.float32).reshape(KC, 128).T.reshape(128, KC, 1))
    vwT = np.ascontiguousarray(
        np.asarray(v_w, dtype=np.float32).reshape(KC, 128).T.reshape(128, KC, 1)
    ).astype(BF16)
    maskadd = ((np.asarray(mask, dtype=np.float32) - 1.0) * 1.0e10).astype(BF16)

    in_maps = []
    for c in range(N_CORES):
        s = slice(c * BC, (c + 1) * BC)
        encT = np.ascontiguousarray(                           # [BC,128,NCHUNK,EC,TCH]
            enc_q[s].reshape(BC, NCHUNK, TCH, EC, 128).transpose(0, 4, 1, 3, 2))
        decT = np.ascontiguousarray(                           # [128, KC, BC]
            np.clip(dec[s].T, -240.0, 240.0).astype(E4M3)
            .reshape(KC, 128, BC).transpose(1, 0, 2))
        in_maps.append(
            {
                "encT": encT,
                "wpair": wpair,
                "wv": wv,
                "decT": decT,
                "attn_bT": bT,
                "v_wT": vwT,
                "maskadd": maskadd[s],
            }
        )
    return in_maps


def run(inputs, trace=False):
    nc = _get_nc()
    in_maps = prepare_in_maps(**inputs)
    res = run_bass_kernel_spmd(nc, in_maps, core_ids=list(range(N_CORES)), trace=trace)
    out = np.concatenate([res.results[c]["out"] for c in range(N_CORES)], axis=0)
    return out.astype(np.float32), res


def kernel(**inputs):
    out, _ = run(inputs, trace=False)
    return out
